# revision 51
# baseline (speedup 1.0000x reference)
"""GraphSAGE supervised forward on 8 Trainium2 NeuronCores.

Full inputs in, full output out. Data-parallel over the B=1024 seed nodes:
128 seeds per core; neighbor rows shard as contiguous row ranges. Tiny
weights replicated.

Design — quantize + transpose on host, PE group-sums, algebraic fold:
  - hop-2 neighbors (the 82MB/core f32 stream) are sent as fp8e4m3 in
    feat-major, PHASE-MAJOR-per-tile layout; hop-1 neighbors fp8;
    seeds fp16. End-to-end max rel err ~8.7e-3 (gate 2e-2): the two
    mean-over-25 stages attenuate per-element quantization noise.
  - group-sum of 25 phases runs on the PE as accumulating identity
    matmuls: stationary [I;I] fp8 + DoubleRow packs 2 phases per
    column-slot (12 DR + 1 plain matmul per tile), f32 PSUM.
  - algebraic fold #1: the hop-1 mean commutes with the aggregator
    matmul -> only per-seed sums survive (redS = DVE reduce of GS PSUM,
    negS = chunked DVE group-sum of negT). fold #2: hop-1 + MLP layer 1
    are linear in seedT/negS/redS -> six host-folded fp16 weight
    products feed one six-matmul PSUM group per part. fold #3: 2-class
    softmax == sigmoid of host-folded logit-difference weights (exact),
    one ACT op; output is class-major [2, BL] so stores are 2-descriptor
    DMAs (host transposes after gather).
DMA orchestration (the kernel is HBM-DMA-bound at ~420 GB/s/core):
  - ALL loads on the two HWDGE rings, NONE on SWDGE: stream tiles issue
    from the sync ring; the 3 packed const tensors (ident2 | negT pack |
    seeds+all-weights pack) from the scalar ring, which also carries the
    final store. SWDGE's Q7 descriptor path is slow to issue (~0.7us/op
    serial), starves behind HWDGE traffic, and its SBUF descriptor-ring
    port contention slows SDMA engine 15 by ~15%.
  - stream bufs=16 (~20MB SBUF) keeps effectively the whole stream in
    flight so no DMA issue ever gates on straggler-delayed completions;
    both HWDGE queues share one completion-sem pool, so consts are
    packed into 3 DMAs to keep pool-reuse waits off the stream.
  - ragged tile taper [...400,300,200,100,100]: one SDMA engine (idx 15)
    can run ~15% slow (run-to-run), serializing the last tiles in a
    per-tile staircase; taper steps stay under the ~3.4us PE idle window
    (half-clock re-throttle) and shrink the final data dependency. Taper
    tiles load as two phase-half DMAs so the first 6 DR matmuls overlap
    the second half's drain (region-level dependency tracking).
  - MLP runs in 3 parts ([0,52),[52,84),[84,128)) as seed ranges
    complete, split into single-stage pieces drained between group-sum
    bursts so the in-order PE queue never sits behind a dependent chain;
    the last part's seed/neg l1 matmuls run mid-stream into a held PSUM
    bank, leaving only 2 redS matmuls + 4-stage chain + one 2-descriptor
    store after the final tile lands. All parts' sigmoids write one
    persistent [2,BL] tile; the single store (scalar ring, issued by ACT
    right after its own final sigmoid) replaces per-part stores, so
    SWDGE is never used and its Q7 drain leaves the end barrier path.
    Warmup matmuls hold the PE clock through fill/tail.
Measured (exec window): 72.5-74.3us on clean runs, ~79-84us under
degraded DMA (engine-15 straggler or uniform slowdown; prior session's
83.1us baseline had the same environmental split).
Fixed costs in the window: ~2.3us pre-stream, ~9us framework epilogue
(semaphore-clear storm), ~1.5us final store issue+HBM receipt.
Falsified variants: big-first tiles (PE starts late, re-throttles,
13-17us tail) and extra early warmups (displace GS work; +5us on
straggler runs) — see session notes before re-trying either.
"""

import sys

for _p in ("/opt/trn_rl_repo", "/root/.axon_site/_ro/trn_rl_repo"):
    if _p not in sys.path:
        sys.path.append(_p)

import numpy as np
import ml_dtypes
from contextlib import ExitStack

import concourse.bass as bass
import concourse.tile as tile
from concourse import bacc, mybir
from concourse.bass_utils import run_bass_kernel_spmd

B, S, D = 1024, 25, 128
NCORES = 8
BL = B // NCORES          # 128 seeds per core
G1 = BL * S               # 3200 hop-1 rows (= hop-2 groups) per core
G2 = G1 * S               # 80000 hop-2 rows per core

# ragged stream tiles (groups per tile, per side); sum = G1.
# 400g mid-tiles keep the 13-matmul-per-tile PE overhead low; the
# 300/200/100/100 taper keeps a straggler DMA engine's per-tile staircase
# step under the ~3.4us PE idle window (half-clock re-throttle), and the
# final 100g tile minimizes the last data dependency
SIZES = [100, 400, 400, 400, 400, 400, 400, 300, 200, 100, 100]
OFFS = np.cumsum([0] + SIZES).tolist()
NTT = len(SIZES)
assert OFFS[-1] == G1 and all(sz % S == 0 for sz in SIZES)

F32 = mybir.dt.float32
F16 = mybir.dt.float16
BF16 = mybir.dt.bfloat16
F8 = mybir.dt.float8e4
AX = mybir.AxisListType
AF = mybir.ActivationFunctionType
DR = mybir.MatmulPerfMode.DoubleRow

NPF8 = ml_dtypes.float8_e4m3
NPBF = ml_dtypes.bfloat16
NPF16 = np.float16


def _build_program():
    nc = bacc.Bacc("TRN2", target_bir_lowering=False, debug=False)

    ins = {}
    for side in ("s", "d"):
        ins[f"nn_{side}"] = nc.dram_tensor(f"nn_{side}", [D, G2], F8, kind="ExternalInput")
    # consts packed into 3 tensors -> 3 DMAs: fewer HWDGE issues, and fewer
    # turns of the (shared!) HWDGE completion-sem pool that would otherwise
    # cross-couple the sync stream ring with the scalar const ring
    ins["ident2"] = nc.dram_tensor("ident2", [D, 2 * D], F8, kind="ExternalInput")
    ins["negpk"] = nc.dram_tensor("negpk", [D, 2 * G1], F8, kind="ExternalInput")
    # f16 pack cols: seedT_s [0:128], seedT_d [128:256], wf 6x128 [256:1024],
    # w2m [1024:1088], w3m on partitions 0:64 [1088:1096], w4m parts 0:8 [1096:1098]
    ins["c16"] = nc.dram_tensor("c16", [D, 1098], F16, kind="ExternalInput")
    # class-major [2, BL]: part stores become [2, w] slices = 2 DMA
    # descriptors instead of w, cutting the final store issue from the
    # kernel-end chain; host transposes after gather
    out_dram = nc.dram_tensor("out", [2, BL], F32, kind="ExternalOutput")

    with tile.TileContext(nc) as tc, ExitStack() as ctx:
        const = ctx.enter_context(tc.tile_pool(name="const", bufs=1))
        persist = ctx.enter_context(tc.tile_pool(name="persist", bufs=1))
        stream = ctx.enter_context(tc.tile_pool(name="stream", bufs=16))
        work = ctx.enter_context(tc.tile_pool(name="work", bufs=3))
        psA = ctx.enter_context(tc.tile_pool(name="psA", bufs=4, space="PSUM"))
        psM = ctx.enter_context(tc.tile_pool(name="psM", bufs=2, space="PSUM"))
        psW = ctx.enter_context(tc.tile_pool(name="psW", bufs=1, space="PSUM"))

        def load_const(name, shape, dt):
            # scalar-ring HWDGE: parallel to the sync stream ring, and avoids
            # the serialized ~0.7us-per-op SWDGE Q7 emission + starved drain
            t = const.tile(shape, dt, tag=name, name=name)
            nc.scalar.dma_start(t[:], ins[name].ap())
            return t

        # order matters: ident2 (alone, tiny) feeds the first stream tile's
        # matmuls; the negT pack goes next so the t=1..4 negS reduces (and
        # the PE pieces behind them) never wait; weights land well before
        # the first l1 piece
        ident2 = load_const("ident2", [D, 2 * D], F8)
        negpk = load_const("negpk", [D, 2 * G1], F8)
        c16 = load_const("c16", [D, 1098], F16)
        negT = {"s": negpk[:, 0:G1], "d": negpk[:, G1:2 * G1]}
        seedT = {"s": c16[:, 0:BL], "d": c16[:, BL:2 * BL]}
        wf = {}
        for i, (side, what) in enumerate(
            (s, w) for s in ("s", "d") for w in ("seed", "neg", "red")
        ):
            wf[side, what] = c16[:, 2 * BL + i * D:2 * BL + (i + 1) * D]
        w2m = c16[:, 1024:1088]
        w3m = c16[0:64, 1088:1096]
        w4m = c16[0:8, 1096:1098]

        idv2 = ident2.rearrange("p (j m) -> p j m", j=2)  # [128, 2, 128]
        id1 = ident2[:, 0:D]                              # [128, 128]

        negS, redS = {}, {}
        for side in ("s", "d"):
            negS[side] = persist.tile([D, BL], F16, tag=f"negS_{side}", name=f"negS_{side}")
            redS[side] = persist.tile([D, BL], F16, tag=f"redS_{side}", name=f"redS_{side}")
        # all parts' sigmoid outputs land in one persistent tile; a single
        # 2-descriptor store after the last sigmoid replaces per-part stores
        # (no SWDGE at all -> its end-of-kernel Q7 drain leaves the barrier path)
        oall = persist.tile([2, BL], F32, tag="oall", name="oall")

        # 3 parts: the last two seed ranges gate only one tiny tile apart,
        # so one merged final part saves a whole serialized MLP chain
        PARTS = [(0, 52), (52, 84), (84, BL)]

        mst = {}
        psL = [None]

        def mlp_l1_early():
            # last part's seed/neg l1 matmuls run mid-stream (negS is final
            # after t=4); only the 2 redS matmuls stay on the tail chain
            lo, hi = PARTS[-1]
            ps1 = psW.tile([D, hi - lo], F32, tag="ps_last")
            srcs = [("s", "seed", seedT["s"]), ("s", "neg", negS["s"]),
                    ("d", "seed", seedT["d"]), ("d", "neg", negS["d"])]
            for i, (side, what, ten) in enumerate(srcs):
                nc.tensor.matmul(ps1[:], wf[side, what][:], ten[:, lo:hi],
                                 start=(i == 0), stop=False)
            psL[0] = ps1

        def mlp_l1(pi):
            # fused hop-1 + MLP layer 1: x@W1 is linear in seedT/negS/redS,
            # so six host-precomputed weight products feed one psum group
            lo, hi = PARTS[pi]
            w = hi - lo
            if pi == len(PARTS) - 1:
                ps1 = psL[0]
                srcs = [("s", "red", redS["s"]), ("d", "red", redS["d"])]
                for i, (side, what, ten) in enumerate(srcs):
                    nc.tensor.matmul(ps1[:], wf[side, what][:], ten[:, lo:hi],
                                     start=False, stop=(i == len(srcs) - 1))
            else:
                ps1 = psM.tile([D, w], F32, tag="ps_m")
                srcs = [("s", "seed", seedT["s"]), ("s", "neg", negS["s"]), ("s", "red", redS["s"]),
                        ("d", "seed", seedT["d"]), ("d", "neg", negS["d"]), ("d", "red", redS["d"])]
                for i, (side, what, ten) in enumerate(srcs):
                    nc.tensor.matmul(ps1[:], wf[side, what][:], ten[:, lo:hi],
                                     start=(i == 0), stop=(i == len(srcs) - 1))
            h1 = work.tile([D, w], F16, tag="h1")
            nc.scalar.activation(h1[:], ps1[:], AF.Relu)
            mst[pi, 1] = h1

        def mlp_l2(pi):
            lo, hi = PARTS[pi]
            w = hi - lo
            ps2 = psM.tile([64, w], F32, tag="ps_m")
            nc.tensor.matmul(ps2[:], w2m[:], mst[pi, 1][:])
            h2 = work.tile([64, w], F16, tag="h2")
            nc.scalar.activation(h2[:], ps2[:], AF.Relu)
            mst[pi, 2] = h2

        def mlp_l3(pi):
            lo, hi = PARTS[pi]
            w = hi - lo
            ps3 = psM.tile([8, w], F32, tag="ps_m")
            nc.tensor.matmul(ps3[:], w3m[:], mst[pi, 2][:])
            h3 = work.tile([8, w], F16, tag="h3")
            nc.scalar.activation(h3[:], ps3[:], AF.Relu)
            mst[pi, 3] = h3

        def mlp_sm(pi):
            # W4 is host-folded to [w4[:,0]-w4[:,1], w4[:,1]-w4[:,0]], so
            # 2-class softmax == elementwise sigmoid of the logit diffs:
            # exact, and one ACT op instead of a 5-op DVE/ACT chain
            lo, hi = PARTS[pi]
            w = hi - lo
            ps4 = psM.tile([2, w], F32, tag="ps_m")
            nc.tensor.matmul(ps4[:], w4m[:], mst[pi, 3][:])
            nc.scalar.activation(oall[:, lo:hi], ps4[:], AF.Sigmoid)
            if pi == len(PARTS) - 1:
                # one store for all parts, on the scalar ring: ACT issues it
                # right after its own final sigmoid (no cross-engine hop)
                nc.scalar.dma_start(out_dram.ap()[:], oall[:], single_packet=True)

        def warmup(n):
            # throwaway fp8 matmuls to hold the PE clock up through DMA
            # stalls in the fill phase; results land in a scratch psum
            for _ in range(n):
                pw = psW.tile([D, 2 * D], F32, tag="ps_warm")
                nc.tensor.matmul(pw[:], id1, ident2[:], start=True, stop=True)

        seeds_done = {"s": 0, "d": 0}
        next_part = [0]
        pieces = []

        def maybe_parts():
            # enqueue part piece-groups once both sides' seed sums reach a
            # boundary; one group (internally independent) drains per stream
            # tile so the in-order PE queue never sits behind a dependent
            # chain
            while next_part[0] < len(PARTS) and min(seeds_done.values()) >= PARTS[next_part[0]][1]:
                pi = next_part[0]
                pieces.extend([
                    [lambda p=pi: mlp_l1(p)],
                    [lambda p=pi: mlp_l2(p)],
                    [lambda p=pi: mlp_l3(p)],
                    [lambda p=pi: mlp_sm(p)],
                ])
                next_part[0] += 1

        def drain_pieces(k):
            for _ in range(k):
                if pieces:
                    for f in pieces.pop(0):
                        f()

        def stream_tile(side, t):
            g0, sz = OFFS[t], SIZES[t]
            xt = stream.tile([D, sz * S], F8, tag="xt", name="xt")
            # all stream tiles on the sync ring: the scalar sequencer must
            # stay free for part-chain ACTIVATEs (a gated DMA issue there
            # head-of-line blocks them), and store sem-waits must not block
            # load issues
            base = g0 * S
            if sz <= 300:
                # taper tiles split into phase-halves (contiguous in the
                # phase-major layout): the first 6 DR matmuls start when
                # half-A lands, overlapping trailing group-sums with the
                # final data drain and halving straggler staircase steps
                h = 12 * sz
                nc.sync.dma_start(xt[:, 0:h], ins[f"nn_{side}"].ap()[:, base:base + h])
                nc.sync.dma_start(
                    xt[:, h:sz * S], ins[f"nn_{side}"].ap()[:, base + h:base + sz * S]
                )
            else:
                nc.sync.dma_start(xt[:], ins[f"nn_{side}"].ap()[:, base:base + sz * S])
            # phase-major tile: xr[:, k, :] = phase k's sz group-columns
            xr = xt.rearrange("p (k g) -> p k g", k=S)
            ps = psA.tile([D, sz], F32, tag="ps_red")
            for i in range(S // 2):
                nc.tensor.matmul(
                    ps[:], idv2, xr[:, 2 * i:2 * i + 2, :],
                    start=(i == 0), stop=False, perf_mode=DR,
                )
            nc.tensor.matmul(ps[:], id1, xr[:, S - 1, :], start=False, stop=True)
            # per-seed sums straight from PSUM (25 group-cols per seed);
            # fp16 out: one rounding of an f32 sum, feeds the 5x-attenuated
            # mean half of hop-1
            with nc.allow_low_precision(reason="fp16 out of f32 psum sums"):
                nc.vector.reduce_sum(
                    redS[side][:, g0 // S:(g0 + sz) // S],
                    ps.rearrange("p (b s) -> p b s", s=S),
                    axis=AX.X,
                )
            seeds_done[side] = (g0 + sz) // S

        # warmup counts after each early tile's group-sum, tuned to the
        # measured DMA fill schedule (each unit ~256 cols of dummy matmul);
        # more warmups here measurably hurt straggler-engine runs — they
        # displace real GS work exactly when data is scarcest
        WARM = {("d", 0): 5}
        for t in range(NTT):
            for side in ("s", "d"):
                stream_tile(side, t)
                if 1 <= t <= 4:
                    # negS in small chunks so the DVE queue never delays the
                    # redS reduces that recycle psA buffers
                    c0 = 2 * (t - 1)
                    with nc.allow_low_precision(reason="fp16 out of fp8 sums"):
                        nc.vector.reduce_sum(
                            negS[side][:, c0 * 16:(c0 + 2) * 16],
                            negT[side][:, c0 * 400:(c0 + 2) * 400]
                            .rearrange("p (b s) -> p b s", s=S),
                            axis=AX.X,
                        )
                if t == 6 and side == "d":
                    mlp_l1_early()
                if t >= 8:
                    # one tiny matmul per trailing tile: feeds the PE HAM
                    # activity window through the straggler staircase
                    warmup(1)
                warmup(WARM.get((side, t), 0))
                maybe_parts()
                # drain faster near the end so only the last part's short
                # chain remains after the final stream tile lands
                drain_pieces(1 if t < 8 else 2)
        while pieces:
            drain_pieces(1)

    nc.compile()
    return nc


_NC_CACHE = None


def _get_program():
    global _NC_CACHE
    if _NC_CACHE is None:
        _NC_CACHE = _build_program()
    return _NC_CACHE


def kernel(src, src_neg, src_neg_neg, dst, dst_neg, dst_neg_neg, w2, W1, W2, W3, W4,
           _trace=False, **trace_kwargs):
    nc = _get_program()

    w2 = np.asarray(w2, np.float32)
    W1 = np.asarray(W1, np.float32)
    wtop = np.ascontiguousarray(w2[:D])
    wbot = np.ascontiguousarray(w2[D:]) / np.float32(S)
    eye = np.eye(D, dtype=np.float32)
    wtb = wtop @ wbot
    wbb = wbot @ wbot
    W4f = np.asarray(W4, np.float32)

    # f16 const pack [D, 1098]: seeds | 6 folded l1 weights | W2 | W3 | W4'
    # (per-core seeds at cols 0:256; replicated weights after)
    wcols = np.zeros((D, 1098 - 2 * BL), np.float32)
    for i, m in enumerate((wtop @ W1[:D], wtb @ W1[:D], wbb @ W1[:D],
                           wtop @ W1[D:], wtb @ W1[D:], wbb @ W1[D:])):
        wcols[:, i * D:(i + 1) * D] = m
    wcols[:, 768:832] = np.asarray(W2, np.float32)
    wcols[0:64, 832:840] = np.asarray(W3, np.float32)
    # sigmoid-softmax fold: softmax([l0,l1]) == sigmoid([l0-l1, l1-l0])
    wcols[0:8, 840:842] = np.stack(
        [W4f[:, 0] - W4f[:, 1], W4f[:, 1] - W4f[:, 0]], axis=1
    )
    wcols16 = wcols.astype(NPF16)
    rep = {
        "ident2": np.concatenate([eye, eye], axis=1).astype(NPF8),
    }

    def shardT(x, dt, rows):
        # [NCORES*rows, D] -> transposed per core -> [NCORES, D, rows]
        return np.ascontiguousarray(
            np.asarray(x).astype(dt).reshape(NCORES, rows, D).transpose(0, 2, 1)
        )

    def shard_nn(x):
        # [NCORES*G2, D] -> fp8, feat-major + phase-major per ragged tile:
        # out[c, f, OFFS[t]*S + k*SIZES[t] + g] = x[c*G2 + (OFFS[t]+g)*S + k, f]
        x8 = np.asarray(x).astype(NPF8).reshape(NCORES, G1, S, D)
        out = np.empty((NCORES, D, G2), NPF8)
        for t, sz in enumerate(SIZES):
            g0 = OFFS[t]
            blk = x8[:, g0:g0 + sz]                    # [C, sz, S, D]
            out[:, :, g0 * S:(g0 + sz) * S] = (
                blk.transpose(0, 3, 2, 1).reshape(NCORES, D, sz * S)
            )
        return out

    negpk = np.concatenate(
        [shardT(src_neg, NPF8, G1), shardT(dst_neg, NPF8, G1)], axis=2
    )
    c16 = np.empty((NCORES, D, 1098), NPF16)
    c16[:, :, 0:BL] = shardT(src, NPF16, BL)
    c16[:, :, BL:2 * BL] = shardT(dst, NPF16, BL)
    c16[:, :, 2 * BL:] = wcols16[None]
    big = {
        "nn_s": shard_nn(src_neg_neg),
        "nn_d": shard_nn(dst_neg_neg),
        "negpk": negpk,
        "c16": c16,
    }
    in_maps = []
    for c in range(NCORES):
        m = dict(rep)
        for k, v in big.items():
            m[k] = v[c]
        in_maps.append(m)

    res = run_bass_kernel_spmd(
        nc, in_maps, list(range(NCORES)), trace=_trace, **trace_kwargs
    )
    # device emits class-major [2, BL] per core; transpose back on host
    out = np.concatenate(
        [res.results[c]["out"].T for c in range(NCORES)], axis=0
    )
    if _trace:
        return out, res
    return out

